# revision 1
# baseline (speedup 1.0000x reference)
"""Trainium2 Bass kernel for nn_ConditionalFeedForward (MoE top-2 routing).

Strategy: expert-parallel across the 8 NeuronCores — core e owns expert e's
weights. Host (numpy) gathers each expert's routed tokens (multi-hot
routing_map), pads to a common capacity CAP, and pre-transposes/pre-tiles
operands into PE-friendly layouts. Each core computes, for its expert:

    hT = silu(w1 @ xT) * (w3 @ xT)          # [FFN, CAP] staged via DRAM
    yT = w2 @ hT                            # [DIM, CAP]

with float32r matmuls (full PE rate, ~1e-4 relative error), SwiGLU fused on
ScalarE (Silu) + VectorE (mul). Host scatter-adds gate-weighted outputs back
to the full [N_TOKENS, DIM] result.
"""

import os
import numpy as np

import concourse.bacc as bacc
import concourse.mybir as mybir
import concourse.tile as tile
from concourse.bass_utils import run_bass_kernel_spmd

# Problem constants (hardcoded per harness contract)
NUM_EXPERTS = 8
DIM = 2048
FFN = 5632
N_CORES = 8
KD = DIM // 128    # 16 contraction tiles for GEMM1/3, output tiles for GEMM2
KF = FFN // 128    # 44 ffn chunks

F32 = mybir.dt.float32
F32R = mybir.dt.float32r

# Compiled program cache keyed by CAP
_PROGRAMS = {}

# Filled by the last kernel() call when BASS_KERNEL_TRACE=1 (for test.py)
LAST_EXEC_NS = None


def _split(total, hi, lo=256):
    """Split `total` into (offset, size) parts, each size in [lo, hi],
    preferring `hi`-sized parts. Requires total >= lo."""
    assert total >= lo
    parts = []
    rem = total
    while rem > hi:
        take = hi if rem - hi >= lo else rem - lo
        parts.append(take)
        rem -= take
    parts.append(rem)
    out = []
    t0 = 0
    for p in parts:
        out.append((t0, p))
        t0 += p
    return out


def _p1_tiles(cap):
    """Phase-1 token tiles: matmul N in [256,512] for full f32r rate."""
    return _split(cap, 512)


def _p2_blocks(cap):
    """Phase-2 token blocks of <=768 (SBUF-resident hT slab per block).
    Largest first: only the smaller later blocks' hT reloads are exposed
    at block boundaries (the slab slot serializes load-after-compute)."""
    return _split(cap, 768)


def _p2_subtiles(bn):
    """Split a block into PSUM-bank-sized matmul N-tiles (each in [256,512])."""
    return _split(bn, 512)


def _build_program(cap):
    nc = bacc.Bacc("TRN2", target_bir_lowering=False, debug=False,
                   num_devices=N_CORES)

    xt_d = nc.dram_tensor("xt", [KD, 128, cap], F32R, kind="ExternalInput")
    w1l_d = nc.dram_tensor("w1l", [KF, 128, KD, 128], F32R, kind="ExternalInput")
    w3l_d = nc.dram_tensor("w3l", [KF, 128, KD, 128], F32R, kind="ExternalInput")
    w2l_d = nc.dram_tensor("w2l", [KD, 128, KF, 128], F32R, kind="ExternalInput")
    yt_d = nc.dram_tensor("yt", [KD, 128, cap], F32, kind="ExternalOutput")
    htb_d = nc.dram_tensor("htb", [KF, 128, cap], F32R, kind="Internal")

    silu = mybir.ActivationFunctionType.Silu
    p1t = _p1_tiles(cap)

    with tile.TileContext(nc) as tc:
        # ---- Phase 1: hT = silu(w1 @ xT) * (w3 @ xT), staged to DRAM ----
        with (
            tc.tile_pool(name="xt", bufs=1) as xpool,
            tc.tile_pool(name="w13", bufs=2) as wpool,
            tc.tile_pool(name="hst", bufs=3) as spool,
            tc.tile_pool(name="ps1", bufs=3, space="PSUM") as psum1,
        ):
            # x loads on the ACT HWDGE ring (idle at kernel start; the SP ring
            # carries the weight stream). Sliced per token-tile so the first
            # PSUM group's 16 k-chunks arrive ASAP.
            # x loads on SWDGE (gpsimd): the SP ring carries the weight
            # stream and the ACT ring carries compute-result stores.
            xt_s = xpool.tile([128, KD, cap], F32R)
            for k in range(KD):
                nc.gpsimd.dma_start(xt_s[:, k, :], xt_d[k])
            for f in range(KF):
                w1c = wpool.tile([128, KD, 128], F32R, tag="w1c")
                nc.sync.dma_start(w1c[:], w1l_d[f])
                w3c = wpool.tile([128, KD, 128], F32R, tag="w3c")
                nc.sync.dma_start(w3c[:], w3l_d[f])
                for (t0, tn) in p1t:
                    h1p = psum1.tile([128, tn], F32, tag="h1p")
                    h3p = psum1.tile([128, tn], F32, tag="h3p")
                    for k in range(KD):
                        nc.tensor.matmul(
                            h1p[:], w1c[:, k, :], xt_s[:, k, t0:t0 + tn],
                            start=(k == 0), stop=(k == KD - 1))
                    for k in range(KD):
                        nc.tensor.matmul(
                            h3p[:], w3c[:, k, :], xt_s[:, k, t0:t0 + tn],
                            start=(k == 0), stop=(k == KD - 1))
                    s1 = spool.tile([128, tn], F32, tag="s1")
                    nc.scalar.activation(s1[:], h1p[:], silu)
                    ht = spool.tile([128, tn], F32, tag="ht")
                    nc.vector.tensor_mul(ht[:], s1[:], h3p[:])
                    nc.scalar.dma_start(htb_d[f][:, t0:t0 + tn],
                                        ht[:].bitcast(F32R))

        # ---- Phase 2: yT = w2 @ hT ----
        with (
            tc.tile_pool(name="htk", bufs=1) as hpool,
            tc.tile_pool(name="w2", bufs=2) as w2pool,
            tc.tile_pool(name="yo", bufs=3) as ypool,
            tc.tile_pool(name="ps2", bufs=4, space="PSUM") as psum2,
        ):
            for (b0, bn) in _p2_blocks(cap):
                # hT reloads on SWDGE: they must not sit behind the weight
                # stream in the SP HWDGE FIFO (they become ready much earlier)
                htk = hpool.tile([128, KF, bn], F32R, tag="htk")
                for k2 in range(KF):
                    nc.gpsimd.dma_start(htk[:, k2, :], htb_d[k2][:, b0:b0 + bn])
                for m in range(KD):
                    # chunk-split so the first matmuls of this m start after
                    # ~1/4 of the weight load instead of all of it
                    w2c = w2pool.tile([128, KF, 128], F32R, tag="w2c")
                    for c in range(4):
                        nc.sync.dma_start(w2c[:, c * 11:(c + 1) * 11, :],
                                          w2l_d[m][:, c * 11:(c + 1) * 11, :])
                    for (s0, sn) in _p2_subtiles(bn):
                        yp = psum2.tile([128, sn], F32, tag="yp")
                        for k2 in range(KF):
                            nc.tensor.matmul(
                                yp[:], w2c[:, k2, :], htk[:, k2, s0:s0 + sn],
                                start=(k2 == 0), stop=(k2 == KF - 1))
                        yo = ypool.tile([128, sn], F32, tag="yo")
                        nc.vector.tensor_copy(yo[:], yp[:])
                        nc.scalar.dma_start(
                            yt_d[m][:, b0 + s0:b0 + s0 + sn], yo[:])

    nc.compile()
    return nc


def kernel(x, expert_indices, expert_weights, w1, w2, w3):
    global LAST_EXEC_NS
    x = np.ascontiguousarray(np.asarray(x, dtype=np.float32))
    routing = np.asarray(expert_indices)
    probs = np.asarray(expert_weights, dtype=np.float32)
    w1 = np.asarray(w1, dtype=np.float32)
    w2 = np.asarray(w2, dtype=np.float32)
    w3 = np.asarray(w3, dtype=np.float32)
    n_tokens = x.shape[0]

    idxs = [np.flatnonzero(routing[:, e]) for e in range(NUM_EXPERTS)]
    max_count = max(len(i) for i in idxs)
    cap = max(512, -(-max_count // 16) * 16)  # round up to multiple of 16
    assert cap <= 2304, f"unexpectedly imbalanced routing: max_count={max_count}"

    if cap not in _PROGRAMS:
        _PROGRAMS[cap] = _build_program(cap)
    nc = _PROGRAMS[cap]

    def _prep(e):
        idx = idxs[e]
        xt = np.zeros((DIM, cap), dtype=np.float32)
        xt[:, :len(idx)] = x[idx].T
        return {
            "xt": xt.reshape(KD, 128, cap),
            # W1L[f,p,k,m] = w1[e][f*128+m, k*128+p]
            "w1l": np.ascontiguousarray(
                w1[e].reshape(KF, 128, KD, 128).transpose(0, 3, 2, 1)),
            "w3l": np.ascontiguousarray(
                w3[e].reshape(KF, 128, KD, 128).transpose(0, 3, 2, 1)),
            # W2L[m,p,k2,d] = w2[e][m*128+d, k2*128+p]
            "w2l": np.ascontiguousarray(
                w2[e].reshape(KD, 128, KF, 128).transpose(0, 3, 2, 1)),
        }

    from concurrent.futures import ThreadPoolExecutor
    with ThreadPoolExecutor(max_workers=NUM_EXPERTS) as pool:
        in_maps = list(pool.map(_prep, range(NUM_EXPERTS)))

    trace = os.environ.get("BASS_KERNEL_TRACE", "0") == "1"
    if trace:
        import importlib.util
        if importlib.util.find_spec("antenv") is None or importlib.util.find_spec(
                "antenv.axon_hooks") is None:
            trace = False  # NTFF hook unavailable in this environment
    res = run_bass_kernel_spmd(
        nc, in_maps, core_ids=list(range(N_CORES)),
        trace=trace, trace_cores=list(range(N_CORES)) if trace else None,
    )
    LAST_EXEC_NS = res.exec_time_ns

    out = np.zeros((n_tokens, DIM), dtype=np.float32)
    for e in range(NUM_EXPERTS):
        idx = idxs[e]
        y_t = res.results[e]["yt"].reshape(DIM, cap)[:, :len(idx)]
        out[idx] += probs[idx, e][:, None] * y_t.T
    return out



# revision 2
# speedup vs baseline: 1.5126x; 1.5126x over previous
"""Trainium2 Bass kernel for nn_ConditionalFeedForward (MoE top-2 routing).

Strategy: expert-parallel across the 8 NeuronCores — core e owns expert e's
weights. Host (numpy) gathers each expert's routed tokens (multi-hot
routing_map), pads to a common capacity CAP, and pre-quantizes operands to a
two-level fp8-e4m3 representation (hi + lo residual, SAME power-of-2 scale so
all product terms share one PSUM scale). Each core computes, for its expert:

    hT = silu(w1 @ xT) * (w3 @ xT)          # [FFN, CAP], SBUF-resident fp8
    yT = w2 @ hT                            # [DIM, CAP]

Every GEMM runs on the PE in fp8 DoubleRow mode (2 k-chunks per instruction,
0.5 cycles/row = 4x the f32r rate) with 3 compensation terms sharing one PSUM
accumulation group:

    W @ X ~= W_hi@X_hi + W_lo@X_hi + W_hi@X_lo        (drop W_lo@X_lo ~ 1e-3)

The cross terms pair (W_lo_k, X_hi_k) and (W_hi_k, X_lo_k) as the two planes
of a single DoubleRow instruction, so each GEMM costs 3 fp8 K-passes = 0.75x
the one-pass f32r time. End-to-end rel err ~1e-3 (vs 2e-2 budget).

The SwiGLU intermediate h is re-quantized to two-level fp8 on-device
(scalar: silu, vector: scaled-mul + residual-sub, gpsimd: hi cast) and stays
SBUF-resident — no DRAM staging between the two phases. Host scatter-adds
gate-weighted outputs back to the full [N_TOKENS, DIM] result.
"""

import os
import numpy as np
import ml_dtypes

import concourse.bacc as bacc
import concourse.mybir as mybir
import concourse.tile as tile
from concourse.bass_utils import run_bass_kernel_spmd

# Problem constants (hardcoded per harness contract)
NUM_EXPERTS = 8
DIM = 2048
FFN = 5632
N_CORES = 8
KD = DIM // 128    # 16 contraction chunks for GEMM1/3; output chunks for GEMM2
KF = FFN // 128    # 44 ffn chunks

F32 = mybir.dt.float32
FP8 = mybir.dt.float8e4
NP_FP8 = ml_dtypes.float8_e4m3
DR = mybir.MatmulPerfMode.DoubleRow

# Power-of-2 quantization scales. fp8 relative precision is scale-free; the
# scale only positions the distribution inside e4m3's normal range
# (2^-6 .. 240). hi and lo share the scale so all matmul terms accumulate in
# one PSUM group.
S_X = 16.0       # x ~ N(0,1): max|x|*16 ~ 90
S_W = 1024.0     # w ~ 0.02*N(0,1): max|w|*1024 ~ 115
S_H = 4.0        # h = silu(g1)*g3, |h| <~ 20: max|h|*4 ~ 80
INV_SXW = 1.0 / (S_X * S_W)          # PSUM -> g dequant (2^-14)
G3_TO_HT = S_H / (S_X * S_W)         # PSUM g3 -> g3*S_H   (2^-12)
INV_SWH = 1.0 / (S_W * S_H)          # PSUM -> y dequant   (2^-12)

# Compiled program cache keyed by CAP
_PROGRAMS = {}

# Filled by the last kernel() call when BASS_KERNEL_TRACE=1 (for test.py)
LAST_EXEC_NS = None


def _split(total, hi, lo=256):
    """Split `total` into (offset, size) parts, each size in [lo, hi],
    preferring `hi`-sized parts. Requires total >= lo."""
    assert total >= lo
    parts = []
    rem = total
    while rem > hi:
        take = hi if rem - hi >= lo else rem - lo
        parts.append(take)
        rem -= take
    parts.append(rem)
    out = []
    t0 = 0
    for p in parts:
        out.append((t0, p))
        t0 += p
    return out


def _build_program(cap):
    nc = bacc.Bacc("TRN2", target_bir_lowering=False, debug=False,
                   num_devices=N_CORES)

    xc_d = nc.dram_tensor("xc", [KD, 128, 2, cap], FP8, kind="ExternalInput")
    w1l_d = nc.dram_tensor("w1l", [KF, 128, KD, 2, 128], FP8,
                           kind="ExternalInput")
    w3l_d = nc.dram_tensor("w3l", [KF, 128, KD, 2, 128], FP8,
                           kind="ExternalInput")
    w2l_d = nc.dram_tensor("w2l", [KD, 128, KF, 2, 128], FP8,
                           kind="ExternalInput")
    yt_d = nc.dram_tensor("yt", [KD, 128, cap], F32, kind="ExternalOutput")

    silu = mybir.ActivationFunctionType.Silu
    copyf = mybir.ActivationFunctionType.Copy
    mult = mybir.AluOpType.mult
    sub = mybir.AluOpType.subtract
    p1t = _split(cap, 512)

    with tile.TileContext(nc) as tc:
        with (
            tc.tile_pool(name="xc", bufs=1) as xpool,
            tc.tile_pool(name="hc", bufs=1) as hpool,
            tc.tile_pool(name="w13", bufs=2) as wpool,
            tc.tile_pool(name="w2", bufs=2) as w2pool,
            tc.tile_pool(name="hst", bufs=3) as spool,
            tc.tile_pool(name="yo", bufs=3) as ypool,
            tc.tile_pool(name="ps1", bufs=3, space="PSUM") as psum1,
            tc.tile_pool(name="ps2", bufs=2, space="PSUM") as psum2,
        ):
            # x (hi+lo fp8) loads on SWDGE (gpsimd ring): the SP ring carries
            # the weight stream.
            xc_s = xpool.tile([128, KD, 2, cap], FP8)
            for k in range(KD):
                nc.gpsimd.dma_start(xc_s[:, k], xc_d[k])
            # SBUF-resident two-level fp8 SwiGLU intermediate
            hc = hpool.tile([128, KF, 2, cap], FP8)

            def dr_group(out_ap, wc, xs, nk, t0, tn):
                """3-term compensated GEMM: one PSUM accumulation group of
                nk/2 hi-hi DoubleRow matmuls + nk cross DoubleRow matmuls."""
                for j in range(nk // 2):
                    nc.tensor.matmul(
                        out_ap, wc[:, 2 * j:2 * j + 2, 1, :],
                        xs[:, 2 * j:2 * j + 2, 0, t0:t0 + tn],
                        start=(j == 0), stop=False, perf_mode=DR)
                for k in range(nk):
                    # plane0: W_lo_k x X_hi_k, plane1: W_hi_k x X_lo_k
                    nc.tensor.matmul(
                        out_ap, wc[:, k], xs[:, k, :, t0:t0 + tn],
                        start=False, stop=(k == nk - 1), perf_mode=DR)

            # ---- Phase 1: hT = silu(w1 @ xT) * (w3 @ xT), fp8 in SBUF ----
            for f in range(KF):
                w1c = wpool.tile([128, KD, 2, 128], FP8, tag="w1c")
                nc.sync.dma_start(w1c[:], w1l_d[f])
                w3c = wpool.tile([128, KD, 2, 128], FP8, tag="w3c")
                nc.sync.dma_start(w3c[:], w3l_d[f])
                for (t0, tn) in p1t:
                    h1p = psum1.tile([128, 512], F32, tag="h1p")
                    dr_group(h1p[:, :tn], w1c, xc_s, KD, t0, tn)
                    h3p = psum1.tile([128, 512], F32, tag="h3p")
                    dr_group(h3p[:, :tn], w3c, xc_s, KD, t0, tn)
                    s1 = spool.tile([128, 512], F32, tag="s1")
                    nc.scalar.activation(s1[:, :tn], h1p[:, :tn], silu,
                                         scale=INV_SXW)
                    ht = spool.tile([128, 512], F32, tag="ht")
                    nc.vector.scalar_tensor_tensor(
                        ht[:, :tn], h3p[:, :tn], G3_TO_HT, s1[:, :tn],
                        mult, mult)
                    nc.gpsimd.tensor_copy(hc[:, f, 0, t0:t0 + tn], ht[:, :tn])
                    nc.vector.tensor_tensor(
                        hc[:, f, 1, t0:t0 + tn], ht[:, :tn],
                        hc[:, f, 0, t0:t0 + tn], sub)

            # ---- Phase 2: yT = w2 @ hT ----
            for m in range(KD):
                w2c = w2pool.tile([128, KF, 2, 128], FP8, tag="w2c")
                # chunk-split so the first matmuls of this m start after
                # ~1/4 of the weight load instead of all of it
                for c in range(4):
                    nc.sync.dma_start(w2c[:, c * 11:(c + 1) * 11],
                                      w2l_d[m][:, c * 11:(c + 1) * 11])
                for (t0, tn) in p1t:
                    yp = psum2.tile([128, 512], F32, tag="yp")
                    dr_group(yp[:, :tn], w2c, hc, KF, t0, tn)
                    yo = ypool.tile([128, 512], F32, tag="yo")
                    nc.scalar.activation(yo[:, :tn], yp[:, :tn], copyf,
                                         scale=INV_SWH)
                    nc.scalar.dma_start(yt_d[m][:, t0:t0 + tn], yo[:, :tn])

    nc.compile()
    return nc


def _quant2(a, scale):
    """Two-level e4m3 quantization at a shared power-of-2 scale.
    Returns (hi, lo) fp8 arrays with hi + lo ~= a * scale."""
    s = (a * scale).astype(np.float32)
    hi = s.astype(NP_FP8)
    lo = (s - hi.astype(np.float32)).astype(NP_FP8)
    return hi, lo


def _pack_w(w, scale):
    """[R, C] f32 -> [R//128, 128, C//128, 2, 128] fp8 with
    out[r, p, k, j, m] = Wq_j[r*128+m, k*128+p]; j=0 lo, j=1 hi."""
    R, C = w.shape
    hi, lo = _quant2(w, scale)
    q = np.stack([lo, hi])                       # [2, R, C]
    q = q.reshape(2, R // 128, 128, C // 128, 128)
    return np.ascontiguousarray(q.transpose(1, 4, 3, 0, 2))


def kernel(x, expert_indices, expert_weights, w1, w2, w3):
    global LAST_EXEC_NS
    x = np.ascontiguousarray(np.asarray(x, dtype=np.float32))
    routing = np.asarray(expert_indices)
    probs = np.asarray(expert_weights, dtype=np.float32)
    w1 = np.asarray(w1, dtype=np.float32)
    w2 = np.asarray(w2, dtype=np.float32)
    w3 = np.asarray(w3, dtype=np.float32)
    n_tokens = x.shape[0]

    idxs = [np.flatnonzero(routing[:, e]) for e in range(NUM_EXPERTS)]
    max_count = max(len(i) for i in idxs)
    cap = max(512, -(-max_count // 16) * 16)  # round up to multiple of 16
    assert cap <= 2304, f"unexpectedly imbalanced routing: max_count={max_count}"

    if cap not in _PROGRAMS:
        _PROGRAMS[cap] = _build_program(cap)
    nc = _PROGRAMS[cap]

    def _prep(e):
        idx = idxs[e]
        xt = np.zeros((DIM, cap), dtype=np.float32)
        xt[:, :len(idx)] = x[idx].T
        xhi, xlo = _quant2(xt, S_X)
        xq = np.stack([xhi, xlo])                # [2, DIM, cap]
        xq = xq.reshape(2, KD, 128, cap)
        return {
            "xc": np.ascontiguousarray(xq.transpose(1, 2, 0, 3)),
            "w1l": _pack_w(w1[e], S_W),
            "w3l": _pack_w(w3[e], S_W),
            "w2l": _pack_w(w2[e], S_W),
        }

    from concurrent.futures import ThreadPoolExecutor
    with ThreadPoolExecutor(max_workers=NUM_EXPERTS) as pool:
        in_maps = list(pool.map(_prep, range(NUM_EXPERTS)))

    trace = os.environ.get("BASS_KERNEL_TRACE", "0") == "1"
    if trace:
        import importlib.util
        if importlib.util.find_spec("antenv") is None or importlib.util.find_spec(
                "antenv.axon_hooks") is None:
            trace = False  # NTFF hook unavailable in this environment
    res = run_bass_kernel_spmd(
        nc, in_maps, core_ids=list(range(N_CORES)),
        trace=trace, trace_cores=list(range(N_CORES)) if trace else None,
    )
    LAST_EXEC_NS = res.exec_time_ns

    out = np.zeros((n_tokens, DIM), dtype=np.float32)
    for e in range(NUM_EXPERTS):
        idx = idxs[e]
        y_t = res.results[e]["yt"].reshape(DIM, cap)[:, :len(idx)]
        out[idx] += probs[idx, e][:, None] * y_t.T
    return out


# revision 4
# speedup vs baseline: 1.5231x; 1.0069x over previous
"""Trainium2 Bass kernel for nn_ConditionalFeedForward (MoE top-2 routing).

Strategy: expert-parallel across the 8 NeuronCores — core e owns expert e's
weights. Host (numpy) gathers each expert's routed tokens (multi-hot
routing_map), pads to a common capacity CAP, and pre-quantizes operands to a
two-level fp8-e4m3 representation (hi + lo residual, SAME power-of-2 scale so
all product terms share one PSUM scale). Each core computes, for its expert:

    hT = silu(w1 @ xT) * (w3 @ xT)          # [FFN, CAP], SBUF-resident fp8
    yT = w2 @ hT                            # [DIM, CAP]

Every GEMM runs on the PE in fp8 DoubleRow mode (2 k-chunks per instruction,
0.5 cycles/row = 4x the f32r rate) with 3 compensation terms sharing one PSUM
accumulation group:

    W @ X ~= W_hi@X_hi + W_lo@X_hi + W_hi@X_lo        (drop W_lo@X_lo ~ 1e-3)

The cross terms pair (W_lo_k, X_hi_k) and (W_hi_k, X_lo_k) as the two planes
of a single DoubleRow instruction, so each GEMM costs 3 fp8 K-passes = 0.75x
the one-pass f32r time. End-to-end rel err ~1e-3 (vs 2e-2 budget).

The SwiGLU intermediate h is re-quantized to two-level fp8 on-device
(scalar: silu, vector: scaled-mul + residual-sub, gpsimd: hi cast) and stays
SBUF-resident — no DRAM staging between the two phases. Host scatter-adds
gate-weighted outputs back to the full [N_TOKENS, DIM] result.
"""

import os
import numpy as np
import ml_dtypes

import concourse.bacc as bacc
import concourse.mybir as mybir
import concourse.tile as tile
from concourse.bass_utils import run_bass_kernel_spmd

# Problem constants (hardcoded per harness contract)
NUM_EXPERTS = 8
DIM = 2048
FFN = 5632
N_CORES = 8
KD = DIM // 128    # 16 contraction chunks for GEMM1/3; output chunks for GEMM2
KF = FFN // 128    # 44 ffn chunks

F32 = mybir.dt.float32
FP8 = mybir.dt.float8e4
NP_FP8 = ml_dtypes.float8_e4m3
DR = mybir.MatmulPerfMode.DoubleRow

# Power-of-2 quantization scales. fp8 relative precision is scale-free; the
# scale only positions the distribution inside e4m3's normal range
# (2^-6 .. 240). hi and lo share the scale so all matmul terms accumulate in
# one PSUM group.
S_X = 16.0       # x ~ N(0,1): max|x|*16 ~ 90
S_W = 1024.0     # w ~ 0.02*N(0,1): max|w|*1024 ~ 115
S_H = 4.0        # h = silu(g1)*g3, |h| <~ 20: max|h|*4 ~ 80
INV_SXW = 1.0 / (S_X * S_W)          # PSUM -> g dequant (2^-14)
G3_TO_HT = S_H / (S_X * S_W)         # PSUM g3 -> g3*S_H   (2^-12)
INV_SWH = 1.0 / (S_W * S_H)          # PSUM -> y dequant   (2^-12)

# Compiled program cache keyed by CAP
_PROGRAMS = {}

# Filled by the last kernel() call when BASS_KERNEL_TRACE=1 (for test.py)
LAST_EXEC_NS = None


def _split(total, hi, lo=256):
    """Split `total` into (offset, size) parts, each size in [lo, hi],
    preferring `hi`-sized parts. Requires total >= lo."""
    assert total >= lo
    parts = []
    rem = total
    while rem > hi:
        take = hi if rem - hi >= lo else rem - lo
        parts.append(take)
        rem -= take
    parts.append(rem)
    out = []
    t0 = 0
    for p in parts:
        out.append((t0, p))
        t0 += p
    return out


def _build_program(cap):
    nc = bacc.Bacc("TRN2", target_bir_lowering=False, debug=False,
                   num_devices=N_CORES)

    xc_d = nc.dram_tensor("xc", [KD, 128, 2, cap], FP8, kind="ExternalInput")
    w1l_d = nc.dram_tensor("w1l", [KF, 128, KD, 2, 128], FP8,
                           kind="ExternalInput")
    w3l_d = nc.dram_tensor("w3l", [KF, 128, KD, 2, 128], FP8,
                           kind="ExternalInput")
    w2l_d = nc.dram_tensor("w2l", [KD, 128, KF, 2, 128], FP8,
                           kind="ExternalInput")
    yt_d = nc.dram_tensor("yt", [KD, 128, cap], F32, kind="ExternalOutput")

    silu = mybir.ActivationFunctionType.Silu
    copyf = mybir.ActivationFunctionType.Copy
    mult = mybir.AluOpType.mult
    sub = mybir.AluOpType.subtract
    p1t = _split(cap, 512)

    with tile.TileContext(nc) as tc:
        with (
            tc.tile_pool(name="xc", bufs=1) as xpool,
            tc.tile_pool(name="hc", bufs=1) as hpool,
            tc.tile_pool(name="w13", bufs=3) as wpool,
            tc.tile_pool(name="w2", bufs=2) as w2pool,
            tc.tile_pool(name="hst", bufs=3) as spool,
            tc.tile_pool(name="yo", bufs=3) as ypool,
            tc.tile_pool(name="ps1", bufs=3, space="PSUM") as psum1,
            tc.tile_pool(name="ps2", bufs=2, space="PSUM") as psum2,
        ):
            # x (hi+lo fp8) loads on SWDGE (gpsimd ring): the SP ring carries
            # the weight stream.
            xc_s = xpool.tile([128, KD, 2, cap], FP8)
            for k in range(KD):
                nc.gpsimd.dma_start(xc_s[:, k], xc_d[k])
            # SBUF-resident two-level fp8 SwiGLU intermediate
            hc = hpool.tile([128, KF, 2, cap], FP8)

            def dr_group(out_ap, wc, xs, nk, t0, tn):
                """3-term compensated GEMM: one PSUM accumulation group of
                nk/2 hi-hi DoubleRow matmuls + nk cross DoubleRow matmuls."""
                for j in range(nk // 2):
                    nc.tensor.matmul(
                        out_ap, wc[:, 2 * j:2 * j + 2, 1, :],
                        xs[:, 2 * j:2 * j + 2, 0, t0:t0 + tn],
                        start=(j == 0), stop=False, perf_mode=DR)
                for k in range(nk):
                    # plane0: W_lo_k x X_hi_k, plane1: W_hi_k x X_lo_k
                    nc.tensor.matmul(
                        out_ap, wc[:, k], xs[:, k, :, t0:t0 + tn],
                        start=False, stop=(k == nk - 1), perf_mode=DR)

            # ---- Phase 1: hT = silu(w1 @ xT) * (w3 @ xT), fp8 in SBUF ----
            for f in range(KF):
                # chunked so the first matmuls of an iteration wait on a
                # quarter-tile, not the whole 0.5MB transfer
                w1c = wpool.tile([128, KD, 2, 128], FP8, tag="w1c")
                for c in range(4):
                    nc.sync.dma_start(w1c[:, c * 4:(c + 1) * 4],
                                      w1l_d[f][:, c * 4:(c + 1) * 4])
                w3c = wpool.tile([128, KD, 2, 128], FP8, tag="w3c")
                for c in range(4):
                    nc.sync.dma_start(w3c[:, c * 4:(c + 1) * 4],
                                      w3l_d[f][:, c * 4:(c + 1) * 4])
                for (t0, tn) in p1t:
                    h1p = psum1.tile([128, 512], F32, tag="h1p")
                    dr_group(h1p[:, :tn], w1c, xc_s, KD, t0, tn)
                    h3p = psum1.tile([128, 512], F32, tag="h3p")
                    dr_group(h3p[:, :tn], w3c, xc_s, KD, t0, tn)
                    s1 = spool.tile([128, 512], F32, tag="s1")
                    nc.scalar.activation(s1[:, :tn], h1p[:, :tn], silu,
                                         scale=INV_SXW)
                    ht = spool.tile([128, 512], F32, tag="ht")
                    nc.vector.scalar_tensor_tensor(
                        ht[:, :tn], h3p[:, :tn], G3_TO_HT, s1[:, :tn],
                        mult, mult)
                    nc.gpsimd.tensor_copy(hc[:, f, 0, t0:t0 + tn], ht[:, :tn])
                    nc.vector.tensor_tensor(
                        hc[:, f, 1, t0:t0 + tn], ht[:, :tn],
                        hc[:, f, 0, t0:t0 + tn], sub)

            # ---- Phase 2: yT = w2 @ hT ----
            for m in range(KD):
                w2c = w2pool.tile([128, KF, 2, 128], FP8, tag="w2c")
                # chunk-split so the first matmuls of this m start after
                # ~1/4 of the weight load instead of all of it
                for c in range(4):
                    nc.sync.dma_start(w2c[:, c * 11:(c + 1) * 11],
                                      w2l_d[m][:, c * 11:(c + 1) * 11])
                for (t0, tn) in p1t:
                    yp = psum2.tile([128, 512], F32, tag="yp")
                    dr_group(yp[:, :tn], w2c, hc, KF, t0, tn)
                    yo = ypool.tile([128, 512], F32, tag="yo")
                    nc.scalar.activation(yo[:, :tn], yp[:, :tn], copyf,
                                         scale=INV_SWH)
                    nc.scalar.dma_start(yt_d[m][:, t0:t0 + tn], yo[:, :tn])

    nc.compile()
    return nc


def _quant2(a, scale):
    """Two-level e4m3 quantization at a shared power-of-2 scale.
    Returns (hi, lo) fp8 arrays with hi + lo ~= a * scale."""
    s = (a * scale).astype(np.float32)
    hi = s.astype(NP_FP8)
    lo = (s - hi.astype(np.float32)).astype(NP_FP8)
    return hi, lo


def _pack_w(w, scale):
    """[R, C] f32 -> [R//128, 128, C//128, 2, 128] fp8 with
    out[r, p, k, j, m] = Wq_j[r*128+m, k*128+p]; j=0 lo, j=1 hi."""
    R, C = w.shape
    hi, lo = _quant2(w, scale)
    q = np.stack([lo, hi])                       # [2, R, C]
    q = q.reshape(2, R // 128, 128, C // 128, 128)
    return np.ascontiguousarray(q.transpose(1, 4, 3, 0, 2))


def kernel(x, expert_indices, expert_weights, w1, w2, w3):
    global LAST_EXEC_NS
    x = np.ascontiguousarray(np.asarray(x, dtype=np.float32))
    routing = np.asarray(expert_indices)
    probs = np.asarray(expert_weights, dtype=np.float32)
    w1 = np.asarray(w1, dtype=np.float32)
    w2 = np.asarray(w2, dtype=np.float32)
    w3 = np.asarray(w3, dtype=np.float32)
    n_tokens = x.shape[0]

    idxs = [np.flatnonzero(routing[:, e]) for e in range(NUM_EXPERTS)]
    max_count = max(len(i) for i in idxs)
    cap = max(512, -(-max_count // 16) * 16)  # round up to multiple of 16
    assert cap <= 2304, f"unexpectedly imbalanced routing: max_count={max_count}"

    if cap not in _PROGRAMS:
        _PROGRAMS[cap] = _build_program(cap)
    nc = _PROGRAMS[cap]

    def _prep(e):
        idx = idxs[e]
        xt = np.zeros((DIM, cap), dtype=np.float32)
        xt[:, :len(idx)] = x[idx].T
        xhi, xlo = _quant2(xt, S_X)
        xq = np.stack([xhi, xlo])                # [2, DIM, cap]
        xq = xq.reshape(2, KD, 128, cap)
        return {
            "xc": np.ascontiguousarray(xq.transpose(1, 2, 0, 3)),
            "w1l": _pack_w(w1[e], S_W),
            "w3l": _pack_w(w3[e], S_W),
            "w2l": _pack_w(w2[e], S_W),
        }

    from concurrent.futures import ThreadPoolExecutor
    with ThreadPoolExecutor(max_workers=NUM_EXPERTS) as pool:
        in_maps = list(pool.map(_prep, range(NUM_EXPERTS)))

    trace = os.environ.get("BASS_KERNEL_TRACE", "0") == "1"
    if trace:
        import importlib.util
        if importlib.util.find_spec("antenv") is None or importlib.util.find_spec(
                "antenv.axon_hooks") is None:
            trace = False  # NTFF hook unavailable in this environment
    res = run_bass_kernel_spmd(
        nc, in_maps, core_ids=list(range(N_CORES)),
        trace=trace, trace_cores=list(range(N_CORES)) if trace else None,
    )
    LAST_EXEC_NS = res.exec_time_ns

    out = np.zeros((n_tokens, DIM), dtype=np.float32)
    for e in range(NUM_EXPERTS):
        idx = idxs[e]
        y_t = res.results[e]["yt"].reshape(DIM, cap)[:, :len(idx)]
        out[idx] += probs[idx, e][:, None] * y_t.T
    return out
